# revision 4
# baseline (speedup 1.0000x reference)
"""Trainium2 Bass kernel for nn_Pooler (segment mean pooling).

Full inputs: features [8, 4096, 256] f32, begins/ends [8, 1024] int.
Sharding: one batch row per NeuronCore (8 cores, no communication).

Per-core algorithm (prefix-sum trick, window length <= 32):
  t = q*128 + m.  R[t] = sum of F rows within t's 128-block before t
  (strict within-block prefix, PE triangular matmul -> DRAM table
  R[4097, 256]).  CC[q] = sum of full blocks before q (tiny matmul ->
  DRAM table CC[33, 256]).  Then P[t] = CC[t>>7] + R[t] and each window
  mean is (P[e] - P[b]) / max(e - b, 1), via 4 indirect-DMA gather sets
  (R and CC at begins and ends; q-indices precomputed on host).
"""
import numpy as np

import concourse.tile as tile
from concourse import bacc, mybir
from concourse.bass import IndirectOffsetOnAxis
from concourse.bass_utils import run_bass_kernel_spmd

F32 = mybir.dt.float32
I32 = mybir.dt.int32

T, D, S = 4096, 256, 1024
Q = 32          # superblocks of 128 rows
GENS = 4
CPG = 4         # psum chunks per generation (each chunk = 2 q's = 512 f32)
J = S // 128    # index columns per partition
MAX_W = 32
USE_F32R = False

_CACHE = {}


def _host_constants():
    # col 0 = ones (block sum -> psum partition 0); col c>=1 = strict lower
    # (inclusive prefix of position c-1 -> psum partition c)
    r = np.arange(128)
    l128 = ((r[:, None] < r[None, :]) | (r[None, :] == 0)).astype(np.float32)
    l32 = (np.arange(32)[:, None] < np.arange(32)[None, :]).astype(np.float32)
    return {"l128": l128, "l32": l32}


def _build(use_f32r=USE_F32R):
    mmdt = mybir.dt.float32r if use_f32r else F32
    nc = bacc.Bacc("TRN2", target_bir_lowering=False, debug=False, num_devices=8)
    Fd = nc.dram_tensor("features", [T, D], F32, kind="ExternalInput").ap()
    Bd = nc.dram_tensor("begins", [S], I32, kind="ExternalInput").ap()
    Ed = nc.dram_tensor("ends", [S], I32, kind="ExternalInput").ap()
    QBd = nc.dram_tensor("qb", [S], I32, kind="ExternalInput").ap()
    QEd = nc.dram_tensor("qe", [S], I32, kind="ExternalInput").ap()
    L128d = nc.dram_tensor("l128", [128, 128], F32, kind="ExternalInput").ap()
    L32d = nc.dram_tensor("l32", [32, 32], F32, kind="ExternalInput").ap()
    OUTd = nc.dram_tensor("out", [S, D], F32, kind="ExternalOutput").ap()

    with tile.TileContext(nc) as tc:
        with (
            tc.tile_pool(name="dram", bufs=1, space="DRAM") as dpool,
            tc.tile_pool(name="consts", bufs=1) as cpool,
            tc.tile_pool(name="xin", bufs=6) as xpool,
            tc.tile_pool(name="apsum", bufs=5, space="PSUM") as ppool,
            tc.tile_pool(name="cpsum", bufs=2, space="PSUM") as cpool_ps,
            tc.tile_pool(name="small", bufs=1) as spool,
            tc.tile_pool(name="gath", bufs=1) as gpool,
        ):
            Rd = dpool.tile([T + 1, D], F32)
            CCd = dpool.tile([Q, D], F32)

            l128 = cpool.tile([128, 128], F32)
            nc.sync.dma_start(l128[:], L128d)
            l32 = cpool.tile([32, 32], F32)
            nc.sync.dma_start(l32[:], L32d)

            idx_sb = {}
            for name, dram in (("b", Bd), ("e", Ed), ("qb", QBd), ("qe", QEd)):
                tl = spool.tile([128, J], I32, name=f"idx_{name}")
                nc.sync.dma_start(tl[:], dram.rearrange("(p j) -> p j", j=J))
                idx_sb[name] = tl

            s32 = spool.tile([32, D], F32)
            srow = spool.tile([1, Q, D], F32)
            zrow = spool.tile([1, D], F32)
            nc.vector.memset(zrow[:], 0.0)
            nc.scalar.dma_start(Rd[0:1, :], zrow[:])

            Fv = Fd.rearrange("(q r) d -> r q d", r=128)
            Rv = Rd[:][:T, :].rearrange("(q m) d -> m q d", m=128)
            # R_dram[q*128 + c] = psum[c] (c>=1); R_dram[128*(q+1)] = psum[0]

            for g in range(GENS):
                for cc in range(CPG):
                    c = g * CPG + cc
                    xt = xpool.tile([128, 2, D], F32)
                    nc.sync.dma_start(xt[:], Fv[:, 2 * c:2 * c + 2, :])
                    pt = ppool.tile([128, 2, D], F32, space="PSUM")
                    nc.tensor.matmul(
                        pt[:],
                        lhsT=l128[:].bitcast(mmdt),
                        rhs=xt[:].bitcast(mmdt),
                        start=True,
                        stop=True,
                    )
                    # block sums land in psum partition 0 (ones column)
                    nc.scalar.copy(srow[0:1, 2 * c:2 * c + 2, :], pt[0:1, :, :])
                    nc.scalar.dma_start(
                        s32[2 * c:2 * c + 2, :], srow[0:1, 2 * c:2 * c + 2, :]
                    )
                    ev = xpool.tile([128, 2, D], F32)
                    nc.vector.tensor_copy(out=ev[:], in_=pt[:])
                    nc.scalar.dma_start(Rv[1:128, 2 * c:2 * c + 2, :], ev[1:128, :, :])
                    for h in range(2):
                        q = 2 * c + h
                        nc.scalar.dma_start(
                            Rd[128 * (q + 1):128 * (q + 1) + 1, :], ev[0:1, h, :]
                        )

            ccp = cpool_ps.tile([32, D], F32, space="PSUM")
            nc.tensor.matmul(
                ccp[:],
                lhsT=l32[:].bitcast(mmdt),
                rhs=s32[:].bitcast(mmdt),
                start=True,
                stop=True,
            )
            cc32 = spool.tile([32, D], F32)
            nc.vector.tensor_copy(out=cc32[:], in_=ccp[:])
            nc.scalar.dma_start(CCd[:], cc32[:])

            rb = gpool.tile([128, J, D], F32)
            re = gpool.tile([128, J, D], F32)
            cb = gpool.tile([128, J, D], F32)
            ce = gpool.tile([128, J, D], F32)
            for j in range(J):
                for out_t, table, idx in (
                    (rb, Rd, "b"), (re, Rd, "e"), (cb, CCd, "qb"), (ce, CCd, "qe"),
                ):
                    nc.gpsimd.indirect_dma_start(
                        out=out_t[:, j, :],
                        out_offset=None,
                        in_=table[:],
                        in_offset=IndirectOffsetOnAxis(
                            ap=idx_sb[idx][:, j:j + 1], axis=0
                        ),
                    )

            dr = gpool.tile([128, J, D], F32)
            nc.vector.tensor_tensor(
                out=dr[:], in0=re[:], in1=rb[:], op=mybir.AluOpType.subtract
            )
            dc = gpool.tile([128, J, D], F32)
            nc.vector.tensor_tensor(
                out=dc[:], in0=ce[:], in1=cb[:], op=mybir.AluOpType.subtract
            )
            diff = gpool.tile([128, J, D], F32)
            nc.vector.tensor_tensor(
                out=diff[:], in0=dr[:], in1=dc[:], op=mybir.AluOpType.add
            )
            cnt = spool.tile([128, J], I32)
            nc.vector.tensor_tensor(
                out=cnt[:], in0=idx_sb["e"][:], in1=idx_sb["b"][:],
                op=mybir.AluOpType.subtract,
            )
            cntf = spool.tile([128, J], F32)
            nc.vector.tensor_copy(out=cntf[:], in_=cnt[:])
            nc.vector.tensor_scalar_max(cntf[:], cntf[:], 1.0)
            rcp = spool.tile([128, J], F32)
            nc.vector.reciprocal(rcp[:], cntf[:])
            res = gpool.tile([128, J, D], F32)
            for j in range(J):
                nc.vector.tensor_scalar_mul(
                    res[:, j, :], diff[:, j, :], rcp[:, j:j + 1]
                )
            nc.sync.dma_start(OUTd.rearrange("(p j) d -> p j d", j=J), res[:])

    nc.compile()
    return nc


def _get_nc():
    if "nc" not in _CACHE:
        _CACHE["nc"] = _build()
        _CACHE["consts"] = _host_constants()
    return _CACHE["nc"], _CACHE["consts"]


def kernel(features, begins, ends, _trace=False, _trace_kwargs=None):
    features = np.ascontiguousarray(np.asarray(features, dtype=np.float32))
    b = np.asarray(begins).astype(np.int64)
    e = np.asarray(ends).astype(np.int64)
    Bn = features.shape[0]
    assert features.shape == (Bn, T, D) and Bn == 8

    # mirror the reference's clamping semantics, then bound windows
    b = np.clip(b, 0, T)
    e = np.clip(e, 0, T)
    e = np.maximum(e, b)           # empty window -> diff 0
    e = np.minimum(e, b + MAX_W)   # reference only sums MAX_W positions
    b32 = b.astype(np.int32)
    e32 = e.astype(np.int32)
    qb32 = np.maximum((b - 1) >> 7, 0).astype(np.int32)
    qe32 = np.maximum((e - 1) >> 7, 0).astype(np.int32)

    nc, consts = _get_nc()
    in_maps = []
    for i in range(Bn):
        m = {
            "features": features[i],
            "begins": np.ascontiguousarray(b32[i]),
            "ends": np.ascontiguousarray(e32[i]),
            "qb": np.ascontiguousarray(qb32[i]),
            "qe": np.ascontiguousarray(qe32[i]),
        }
        m.update(consts)
        in_maps.append(m)
    kw = {}
    if _trace:
        tk = dict(_trace_kwargs or {})
        tmpdir = tk.pop("tmpdir", None)
        kw = {"trace": True, "trace_kwargs": tk}
        if tmpdir:
            kw["tmpdir"] = tmpdir
    res = run_bass_kernel_spmd(nc, in_maps, list(range(Bn)), **kw)
    out = np.stack([res.results[i]["out"] for i in range(Bn)])
    if _trace:
        return out, res
    return out


# revision 7
# speedup vs baseline: 2.0586x; 2.0586x over previous
"""Trainium2 Bass kernel for nn_Pooler (segment mean pooling).

Full inputs: features [8, 4096, 256] f32, begins/ends [8, 1024] int.
Sharding: one batch row per NeuronCore (8 cores, no communication).

Per-core algorithm (prefix-sum trick, window length <= 32):
  t = q*128 + j.  A rotated-triangular PE matmul produces, per block q,
  psum[0] = full block sum and psum[c] = inclusive prefix of rows < c
  (c >= 1).  Features are split on device into exact bf16 hi+lo pairs
  (hi cast on the scalar engine, lo on vector) so the triangular matmul
  runs at bf16 rate while keeping ~2^-16 relative input precision.
  Cross-block offsets CC[q] (tiny fp32 triangular matmul, pipelined one
  generation behind) are broadcast across partitions (4-row DMA + DVE
  stream_shuffle) and added during psum eviction, so the DRAM table
  holds exclusive global prefix sums P directly, in a partition-major
  permuted layout T'[c*32 + q] = P[t] (c = t&127, q = (t-1)>>7; row
  4096 = P[0] = 0) that makes each eviction DMA fully contiguous (8KB
  runs).  Window means are (P[e] - P[b]) * rcp via indirect-DMA row
  gathers; permuted gather indices and reciprocals precomputed on host.
"""
import numpy as np

import concourse.tile as tile
from concourse import bacc, mybir
from concourse.bass import IndirectOffsetOnAxis
from concourse.bass_utils import run_bass_kernel_spmd

F32 = mybir.dt.float32
BF16 = mybir.dt.bfloat16
I32 = mybir.dt.int32

T, D, S = 4096, 256, 1024
Q = 32          # superblocks of 128 rows
GENS = 4
CPG = 4         # psum chunks per generation (each chunk = 2 q's = 512 f32)
J = S // 128    # index columns per partition
MAX_W = 32
SPLIT_BF16 = True

_CACHE = {}


def _host_constants():
    # col 0 = ones (block sum -> psum partition 0); col c>=1 = strict lower
    # (inclusive prefix of rows < c -> psum partition c)
    r = np.arange(128)
    l128 = ((r[:, None] < r[None, :]) | (r[None, :] == 0)).astype(np.float32)
    l32 = (np.arange(32)[:, None] < np.arange(32)[None, :]).astype(np.float32)
    return {"l128": l128, "l32": l32}


def _permute_idx(t):
    c = t & 127
    q = (t - 1) >> 7
    return np.where(t == 0, T, c * Q + q).astype(np.int32)


def _build():
    nc = bacc.Bacc("TRN2", target_bir_lowering=False, debug=False, num_devices=8)
    Fd = nc.dram_tensor("features", [T, D], F32, kind="ExternalInput").ap()
    GBd = nc.dram_tensor("gbi", [S], I32, kind="ExternalInput").ap()
    GEd = nc.dram_tensor("gei", [S], I32, kind="ExternalInput").ap()
    RCPd = nc.dram_tensor("rcp", [S], F32, kind="ExternalInput").ap()
    L128d = nc.dram_tensor("l128", [128, 128], F32, kind="ExternalInput").ap()
    L32d = nc.dram_tensor("l32", [32, 32], F32, kind="ExternalInput").ap()
    OUTd = nc.dram_tensor("out", [S, D], F32, kind="ExternalOutput").ap()

    with tile.TileContext(nc) as tc:
        with (
            tc.tile_pool(name="dram", bufs=1, space="DRAM") as dpool,
            tc.tile_pool(name="consts", bufs=1) as cpool,
            tc.tile_pool(name="xin", bufs=6) as xpool,
            tc.tile_pool(name="xsplit", bufs=6) as hpool,
            tc.tile_pool(name="evg", bufs=2) as epool,
            tc.tile_pool(name="ccrep", bufs=2) as rpool,
            tc.tile_pool(name="apsum", bufs=7, space="PSUM") as ppool,
            tc.tile_pool(name="cpsum", bufs=1, space="PSUM") as cpool_ps,
            tc.tile_pool(name="small", bufs=1) as spool,
            tc.tile_pool(name="gath", bufs=1) as gpool,
        ):
            Td = dpool.tile([T + 1, D], F32)
            Tv = Td[:][:T, :].rearrange("(c q) d -> c q d", q=Q)

            l128 = cpool.tile([128, 128], F32)
            nc.sync.dma_start(l128[:], L128d)
            l128b = cpool.tile([128, 128], BF16)
            nc.vector.tensor_copy(out=l128b[:], in_=l128[:])
            l32 = cpool.tile([32, 32], F32)
            nc.sync.dma_start(l32[:], L32d)

            gbi = spool.tile([128, J], I32)
            nc.sync.dma_start(gbi[:], GBd.rearrange("(p j) -> p j", j=J))
            gei = spool.tile([128, J], I32)
            nc.sync.dma_start(gei[:], GEd.rearrange("(p j) -> p j", j=J))
            rcp = spool.tile([128, J], F32)
            nc.sync.dma_start(rcp[:], RCPd.rearrange("(p j) -> p j", j=J))

            s32 = spool.tile([32, D], F32)
            nc.vector.memset(s32[:], 0.0)
            srow = spool.tile([1, Q, D], F32)
            ccq = spool.tile([128, CPG * 2, D], F32)
            nc.vector.memset(ccq[:], 0.0)
            zrow = spool.tile([1, D], F32)
            nc.vector.memset(zrow[:], 0.0)
            nc.scalar.dma_start(Td[T:T + 1, :], zrow[:])

            Fv = Fd.rearrange("(q r) d -> r q d", r=128)

            def emit_gen_matmuls(g):
                chunk_psums = []
                for cc in range(CPG):
                    c = g * CPG + cc
                    xt = xpool.tile([128, 2, D], F32)
                    nc.sync.dma_start(xt[:], Fv[:, 2 * c:2 * c + 2, :])
                    pt = ppool.tile([128, 2, D], F32, space="PSUM")
                    if SPLIT_BF16:
                        xh = hpool.tile([128, 2, D], BF16)
                        nc.scalar.copy(xh[:], xt[:])
                        xl = hpool.tile([128, 2, D], BF16)
                        nc.vector.tensor_tensor(
                            out=xl[:], in0=xt[:], in1=xh[:],
                            op=mybir.AluOpType.subtract,
                        )
                        nc.tensor.matmul(
                            pt[:], lhsT=l128b[:], rhs=xh[:],
                            start=True, stop=False,
                        )
                        nc.tensor.matmul(
                            pt[:], lhsT=l128b[:], rhs=xl[:],
                            start=False, stop=True,
                        )
                    else:
                        nc.tensor.matmul(
                            pt[:], lhsT=l128[:], rhs=xt[:], start=True, stop=True
                        )
                    # block sums land in psum partition 0 (ones column)
                    nc.scalar.copy(srow[0:1, 2 * c:2 * c + 2, :], pt[0:1, :, :])
                    nc.scalar.dma_start(
                        s32[2 * c:2 * c + 2, :], srow[0:1, 2 * c:2 * c + 2, :]
                    )
                    chunk_psums.append((cc, pt))
                return chunk_psums

            def emit_gen_ccp(g):
                ccp = cpool_ps.tile([8, D], F32, space="PSUM")
                nc.tensor.matmul(
                    ccp[:], lhsT=l32[:, 8 * g:8 * (g + 1)], rhs=s32[:],
                    start=True, stop=True,
                )
                return ccp

            def emit_gen_cc_and_evict(g, chunk_psums, ccp):
                cc8 = spool.tile([8, D], F32, name=f"cc8_{g}")
                nc.vector.tensor_copy(out=cc8[:], in_=ccp[:])
                for p in (0, 32, 64, 96):
                    nc.scalar.dma_start(ccq[p:p + 1, :, :], cc8[:])
                ccrep = rpool.tile([128, CPG * 2, D], F32)
                nc.vector.stream_shuffle(ccrep[:], ccq[:], [0] * 32)

                evg = epool.tile([128, CPG * 2, D], F32)
                for cc, pt in chunk_psums:
                    nc.vector.tensor_tensor(
                        out=evg[:, 2 * cc:2 * cc + 2, :],
                        in0=pt[:],
                        in1=ccrep[:, 2 * cc:2 * cc + 2, :],
                        op=mybir.AluOpType.add,
                    )
                nc.scalar.dma_start(Tv[:, 8 * g:8 * (g + 1), :], evg[:])

            # software pipeline: CC chain for gen g emitted after gen g+1
            # matmuls so the PE stream never stalls on the S extraction
            pending = None
            for g in range(GENS):
                psums = emit_gen_matmuls(g)
                ccp = emit_gen_ccp(g)
                if pending is not None:
                    emit_gen_cc_and_evict(g - 1, *pending)
                pending = (psums, ccp)
            emit_gen_cc_and_evict(GENS - 1, *pending)

            pex = gpool.tile([128, J, D], F32)
            pb = gpool.tile([128, J, D], F32)
            for j in range(J):
                nc.gpsimd.indirect_dma_start(
                    out=pb[:, j, :], out_offset=None, in_=Td[:],
                    in_offset=IndirectOffsetOnAxis(ap=gbi[:, j:j + 1], axis=0),
                )
                nc.gpsimd.indirect_dma_start(
                    out=pex[:, j, :], out_offset=None, in_=Td[:],
                    in_offset=IndirectOffsetOnAxis(ap=gei[:, j:j + 1], axis=0),
                )
            nc.vector.tensor_tensor(
                out=pex[:], in0=pex[:], in1=pb[:], op=mybir.AluOpType.subtract
            )

            res = gpool.tile([128, J, D], F32)
            for j in range(J):
                # scaled copy on the scalar engine: out = in * rcp[p]
                nc.scalar.mul(res[:, j, :], pex[:, j, :], rcp[:, j:j + 1])
            nc.sync.dma_start(OUTd.rearrange("(p j) d -> p j d", j=J), res[:])

    nc.compile()
    return nc


def _get_nc():
    if "nc" not in _CACHE:
        _CACHE["nc"] = _build()
        _CACHE["consts"] = _host_constants()
    return _CACHE["nc"], _CACHE["consts"]


def _prep_indices(b, e):
    b = np.clip(b, 0, T)
    e = np.clip(e, 0, T)
    e = np.maximum(e, b)           # empty window -> diff 0
    e = np.minimum(e, b + MAX_W)   # reference only sums MAX_W positions
    gbi = _permute_idx(b.astype(np.int64))
    gei = _permute_idx(e.astype(np.int64))
    rcp = (1.0 / np.maximum(e - b, 1)).astype(np.float32)
    return gbi, gei, rcp


def kernel(features, begins, ends, _trace=False, _trace_kwargs=None):
    features = np.ascontiguousarray(np.asarray(features, dtype=np.float32))
    b = np.asarray(begins).astype(np.int64)
    e = np.asarray(ends).astype(np.int64)
    Bn = features.shape[0]
    assert features.shape == (Bn, T, D) and Bn == 8

    gbi, gei, rcp = _prep_indices(b, e)

    nc, consts = _get_nc()
    in_maps = []
    for i in range(Bn):
        m = {
            "features": features[i],
            "gbi": np.ascontiguousarray(gbi[i]),
            "gei": np.ascontiguousarray(gei[i]),
            "rcp": np.ascontiguousarray(rcp[i]),
        }
        m.update(consts)
        in_maps.append(m)
    kw = {}
    if _trace:
        tk = dict(_trace_kwargs or {})
        tmpdir = tk.pop("tmpdir", None)
        kw = {"trace": True, "trace_kwargs": tk}
        if tmpdir:
            kw["tmpdir"] = tmpdir
    res = run_bass_kernel_spmd(nc, in_maps, list(range(Bn)), **kw)
    out = np.stack([res.results[i]["out"] for i in range(Bn)])
    if _trace:
        return out, res
    return out


# revision 9
# speedup vs baseline: 2.2690x; 1.1022x over previous
"""Trainium2 Bass kernel for nn_Pooler (segment mean pooling).

Full inputs: features [8, 4096, 256] f32, begins/ends [8, 1024] int.
Sharding: one batch row per NeuronCore (8 cores, no communication).

Per-core algorithm (prefix-sum trick, window length <= 32):
  t = q*128 + j.  A rotated-triangular PE matmul produces, per block q,
  psum[0] = full block sum and psum[c] = inclusive prefix of rows < c
  (c >= 1).  Features are split on device into exact bf16 hi+lo pairs
  so the triangular matmul runs at bf16 rate with ~2^-17 relative input
  precision.  PSUM banks are evicted promptly by plain copies; the
  cross-block offsets CC[q] (tiny fp32 triangular matmul, pipelined one
  generation behind) are broadcast across partitions (4-row DMA + DVE
  stream_shuffle) and added in-place per generation, so the DRAM table
  holds exclusive global prefix sums P directly, in a partition-major
  permuted layout T'[c*32 + q] = P[t] (c = t&127, q = (t-1)>>7; row
  4096 = P[0] = 0) that makes each eviction DMA fully contiguous (8KB
  runs).  Window means are (P[e] - P[b]) * rcp via two multi-index
  indirect-DMA gathers; permuted gather indices and reciprocals are
  precomputed on host.
"""
import numpy as np

import concourse.tile as tile
from concourse import bacc, mybir
from concourse.bass import IndirectOffsetOnAxis
from concourse.bass_utils import run_bass_kernel_spmd

F32 = mybir.dt.float32
BF16 = mybir.dt.bfloat16
I32 = mybir.dt.int32

T, D, S = 4096, 256, 1024
Q = 32          # superblocks of 128 rows
GENS = 4
CPG = 4         # psum chunks per generation (each chunk = 2 q's = 512 f32)
J = S // 128    # index columns per partition
MAX_W = 32
SPLIT_BF16 = True
MULTI_IDX = False

_CACHE = {}


def _host_constants():
    # col 0 = ones (block sum -> psum partition 0); col c>=1 = strict lower
    # (inclusive prefix of rows < c -> psum partition c)
    r = np.arange(128)
    l128 = ((r[:, None] < r[None, :]) | (r[None, :] == 0)).astype(np.float32)
    l32 = (np.arange(32)[:, None] < np.arange(32)[None, :]).astype(np.float32)
    return {"l128": l128, "l32": l32}


def _permute_idx(t):
    c = t & 127
    q = (t - 1) >> 7
    return np.where(t == 0, T, c * Q + q).astype(np.int32)


def _build():
    nc = bacc.Bacc("TRN2", target_bir_lowering=False, debug=False, num_devices=8)
    Fd = nc.dram_tensor("features", [T, D], F32, kind="ExternalInput").ap()
    GBd = nc.dram_tensor("gbi", [S], I32, kind="ExternalInput").ap()
    GEd = nc.dram_tensor("gei", [S], I32, kind="ExternalInput").ap()
    RCPd = nc.dram_tensor("rcp", [S], F32, kind="ExternalInput").ap()
    L128d = nc.dram_tensor("l128", [128, 128], F32, kind="ExternalInput").ap()
    L32d = nc.dram_tensor("l32", [32, 32], F32, kind="ExternalInput").ap()
    OUTd = nc.dram_tensor("out", [S, D], F32, kind="ExternalOutput").ap()

    with tile.TileContext(nc) as tc:
        with (
            tc.tile_pool(name="dram", bufs=1, space="DRAM") as dpool,
            tc.tile_pool(name="consts", bufs=1) as cpool,
            tc.tile_pool(name="xin", bufs=6) as xpool,
            tc.tile_pool(name="xsplit", bufs=6) as hpool,
            tc.tile_pool(name="evg", bufs=2) as epool,
            tc.tile_pool(name="ccrep", bufs=2) as rpool,
            tc.tile_pool(name="apsum", bufs=6, space="PSUM") as ppool,
            tc.tile_pool(name="cpsum", bufs=2, space="PSUM") as cpool_ps,
            tc.tile_pool(name="small", bufs=1) as spool,
            tc.tile_pool(name="gath", bufs=1) as gpool,
        ):
            Td = dpool.tile([T + 1, D], F32)
            Tv = Td[:][:T, :].rearrange("(c q) d -> c q d", q=Q)

            l128 = cpool.tile([128, 128], F32)
            nc.sync.dma_start(l128[:], L128d)
            l128b = cpool.tile([128, 128], BF16)
            nc.vector.tensor_copy(out=l128b[:], in_=l128[:])
            l32 = cpool.tile([32, 32], F32)
            nc.sync.dma_start(l32[:], L32d)

            gbi = spool.tile([128, J], I32)
            nc.sync.dma_start(gbi[:], GBd.rearrange("(p j) -> p j", j=J))
            gei = spool.tile([128, J], I32)
            nc.sync.dma_start(gei[:], GEd.rearrange("(p j) -> p j", j=J))
            rcp = spool.tile([128, J], F32)
            nc.sync.dma_start(rcp[:], RCPd.rearrange("(p j) -> p j", j=J))

            s32 = spool.tile([32, D], F32)
            nc.vector.memset(s32[:], 0.0)
            srow = spool.tile([1, Q, D], F32)
            ccq = spool.tile([128, CPG * 2, D], F32)
            nc.vector.memset(ccq[:], 0.0)
            zrow = spool.tile([1, D], F32)
            nc.vector.memset(zrow[:], 0.0)
            nc.scalar.dma_start(Td[T:T + 1, :], zrow[:])

            Fv = Fd.rearrange("(q r) d -> r q d", r=128)

            def emit_gen_matmuls(g):
                evg = epool.tile([128, CPG * 2, D], F32)
                for cc in range(CPG):
                    c = g * CPG + cc
                    xt = xpool.tile([128, 2, D], F32)
                    nc.sync.dma_start(xt[:], Fv[:, 2 * c:2 * c + 2, :])
                    pt = ppool.tile([128, 2, D], F32, space="PSUM")
                    if SPLIT_BF16:
                        xh = hpool.tile([128, 2, D], BF16)
                        nc.scalar.copy(xh[:], xt[:])
                        xl = hpool.tile([128, 2, D], BF16)
                        nc.vector.tensor_tensor(
                            out=xl[:], in0=xt[:], in1=xh[:],
                            op=mybir.AluOpType.subtract,
                        )
                        nc.tensor.matmul(
                            pt[:], lhsT=l128b[:], rhs=xh[:],
                            start=True, stop=False,
                        )
                        nc.tensor.matmul(
                            pt[:], lhsT=l128b[:], rhs=xl[:],
                            start=False, stop=True,
                        )
                    else:
                        nc.tensor.matmul(
                            pt[:], lhsT=l128[:], rhs=xt[:], start=True, stop=True
                        )
                    # block sums land in psum partition 0 (ones column)
                    nc.scalar.copy(srow[0:1, 2 * c:2 * c + 2, :], pt[0:1, :, :])
                    nc.scalar.dma_start(
                        s32[2 * c:2 * c + 2, :], srow[0:1, 2 * c:2 * c + 2, :]
                    )
                    # prompt eviction frees the psum bank
                    nc.vector.tensor_copy(
                        out=evg[:, 2 * cc:2 * cc + 2, :], in_=pt[:]
                    )
                return evg

            def emit_gen_cc_and_evict(g, evg):
                ccp = cpool_ps.tile([8, D], F32, space="PSUM")
                nc.tensor.matmul(
                    ccp[:], lhsT=l32[:, 8 * g:8 * (g + 1)], rhs=s32[:],
                    start=True, stop=True,
                )
                cc8 = spool.tile([8, D], F32, name=f"cc8_{g}")
                nc.vector.tensor_copy(out=cc8[:], in_=ccp[:])
                for p in (0, 32, 64, 96):
                    nc.scalar.dma_start(ccq[p:p + 1, :, :], cc8[:])
                ccrep = rpool.tile([128, CPG * 2, D], F32)
                nc.vector.stream_shuffle(ccrep[:], ccq[:], [0] * 32)
                nc.vector.tensor_tensor(
                    out=evg[:], in0=evg[:], in1=ccrep[:],
                    op=mybir.AluOpType.add,
                )
                nc.scalar.dma_start(Tv[:, 8 * g:8 * (g + 1), :], evg[:])

            # software pipeline: CC chain for gen g emitted after gen g+1
            # matmuls; prompt copies free psum banks so nothing deadlocks
            pending = None
            for g in range(GENS):
                evg = emit_gen_matmuls(g)
                if pending is not None:
                    emit_gen_cc_and_evict(g - 1, pending)
                pending = evg
            emit_gen_cc_and_evict(GENS - 1, pending)

            pex = gpool.tile([128, J, D], F32)
            pb = gpool.tile([128, J, D], F32)
            if MULTI_IDX:
                nc.gpsimd.indirect_dma_start(
                    out=pb[:], out_offset=None, in_=Td[:],
                    in_offset=IndirectOffsetOnAxis(ap=gbi[:], axis=0),
                )
                nc.gpsimd.indirect_dma_start(
                    out=pex[:], out_offset=None, in_=Td[:],
                    in_offset=IndirectOffsetOnAxis(ap=gei[:], axis=0),
                )
            else:
                for j in range(J):
                    nc.gpsimd.indirect_dma_start(
                        out=pb[:, j, :], out_offset=None, in_=Td[:],
                        in_offset=IndirectOffsetOnAxis(ap=gbi[:, j:j + 1], axis=0),
                    )
                    nc.gpsimd.indirect_dma_start(
                        out=pex[:, j, :], out_offset=None, in_=Td[:],
                        in_offset=IndirectOffsetOnAxis(ap=gei[:, j:j + 1], axis=0),
                    )
            nc.vector.tensor_tensor(
                out=pex[:], in0=pex[:], in1=pb[:], op=mybir.AluOpType.subtract
            )

            res = gpool.tile([128, J, D], F32)
            for j in range(J):
                # scaled copy on the scalar engine: out = in * rcp[p]
                nc.scalar.mul(res[:, j, :], pex[:, j, :], rcp[:, j:j + 1])
            nc.sync.dma_start(OUTd.rearrange("(p j) d -> p j d", j=J), res[:])

    nc.compile()
    return nc


def _get_nc():
    if "nc" not in _CACHE:
        _CACHE["nc"] = _build()
        _CACHE["consts"] = _host_constants()
    return _CACHE["nc"], _CACHE["consts"]


def _prep_indices(b, e):
    b = np.clip(b, 0, T)
    e = np.clip(e, 0, T)
    e = np.maximum(e, b)           # empty window -> diff 0
    e = np.minimum(e, b + MAX_W)   # reference only sums MAX_W positions
    gbi = _permute_idx(b.astype(np.int64))
    gei = _permute_idx(e.astype(np.int64))
    rcp = (1.0 / np.maximum(e - b, 1)).astype(np.float32)
    return gbi, gei, rcp


def kernel(features, begins, ends, _trace=False, _trace_kwargs=None):
    features = np.ascontiguousarray(np.asarray(features, dtype=np.float32))
    b = np.asarray(begins).astype(np.int64)
    e = np.asarray(ends).astype(np.int64)
    Bn = features.shape[0]
    assert features.shape == (Bn, T, D) and Bn == 8

    gbi, gei, rcp = _prep_indices(b, e)

    nc, consts = _get_nc()
    in_maps = []
    for i in range(Bn):
        m = {
            "features": features[i],
            "gbi": np.ascontiguousarray(gbi[i]),
            "gei": np.ascontiguousarray(gei[i]),
            "rcp": np.ascontiguousarray(rcp[i]),
        }
        m.update(consts)
        in_maps.append(m)
    kw = {}
    if _trace:
        tk = dict(_trace_kwargs or {})
        tmpdir = tk.pop("tmpdir", None)
        kw = {"trace": True, "trace_kwargs": tk}
        if tmpdir:
            kw["tmpdir"] = tmpdir
    res = run_bass_kernel_spmd(nc, in_maps, list(range(Bn)), **kw)
    out = np.stack([res.results[i]["out"] for i in range(Bn)])
    if _trace:
        return out, res
    return out


# revision 10
# speedup vs baseline: 2.3295x; 1.0267x over previous
"""Trainium2 Bass kernel for nn_Pooler (segment mean pooling).

Full inputs: features [8, 4096, 256] f32, begins/ends [8, 1024] int.
Sharding: one batch row per NeuronCore (8 cores, no communication).

Per-core algorithm (prefix-sum trick, window length <= 32):
  t = q*128 + j.  A rotated-triangular PE matmul produces, per block q,
  psum[0] = full block sum and psum[c] = inclusive prefix of rows < c
  (c >= 1).  Features are split on device into exact bf16 hi+lo pairs
  so the triangular matmul runs at bf16 rate with ~2^-17 relative input
  precision.  PSUM banks are evicted promptly by plain copies
  (alternating vector/scalar engines); cross-block offsets CC[q] (tiny
  fp32 triangular matmul, pipelined one generation behind) are
  broadcast across partitions with gpsimd partition_broadcast and added
  in-place per generation, so the DRAM table holds exclusive global
  prefix sums P directly, in a partition-major permuted layout
  T'[c*32 + q] = P[t] (c = t&127, q = (t-1)>>7; row 4096 = P[0] = 0)
  that makes each eviction DMA fully contiguous (8KB runs).  Window
  means are (P[e] - P[b]) * rcp via two 1024-index gpsimd dma_gather
  ops; permuted int16 gather indices and reciprocals are precomputed
  on host.
"""
import numpy as np

import concourse.tile as tile
from concourse import bacc, library_config, mybir
from concourse.bass_utils import run_bass_kernel_spmd

F32 = mybir.dt.float32
BF16 = mybir.dt.bfloat16
I16 = mybir.dt.int16

T, D, S = 4096, 256, 1024
Q = 32          # superblocks of 128 rows
GENS = 4
CPG = 4         # psum chunks per generation (each chunk = 2 q's = 512 f32)
J = S // 128    # gather output columns (span s = col*128 + partition)
MAX_W = 32
SPLIT_BF16 = True

_CACHE = {}


def _host_constants():
    # col 0 = ones (block sum -> psum partition 0); col c>=1 = strict lower
    # (inclusive prefix of rows < c -> psum partition c)
    r = np.arange(128)
    l128 = ((r[:, None] < r[None, :]) | (r[None, :] == 0)).astype(np.float32)
    l32 = (np.arange(32)[:, None] < np.arange(32)[None, :]).astype(np.float32)
    return {"l128": l128, "l32": l32}


def _permute_idx(t):
    c = t & 127
    q = (t - 1) >> 7
    return np.where(t == 0, T, c * Q + q).astype(np.int64)


def _wrap_idx16(idx):
    # dma_gather index layout: idx k at partition k%16, column k//16,
    # replicated to all eight 16-partition groups -> [128, S//16] int16
    w = idx.reshape(S // 16, 16).T.astype(np.int16)
    return np.ascontiguousarray(np.tile(w, (8, 1)))


def _build():
    nc = bacc.Bacc("TRN2", target_bir_lowering=False, debug=False, num_devices=8)
    Fd = nc.dram_tensor("features", [T, D], F32, kind="ExternalInput").ap()
    GBd = nc.dram_tensor("gbi", [128, S // 16], I16, kind="ExternalInput").ap()
    GEd = nc.dram_tensor("gei", [128, S // 16], I16, kind="ExternalInput").ap()
    RCPd = nc.dram_tensor("rcp", [S], F32, kind="ExternalInput").ap()
    L128d = nc.dram_tensor("l128", [128, 128], F32, kind="ExternalInput").ap()
    L32d = nc.dram_tensor("l32", [32, 32], F32, kind="ExternalInput").ap()
    OUTd = nc.dram_tensor("out", [S, D], F32, kind="ExternalOutput").ap()

    with tile.TileContext(nc) as tc:
        with (
            tc.tile_pool(name="dram", bufs=1, space="DRAM") as dpool,
            tc.tile_pool(name="consts", bufs=1) as cpool,
            tc.tile_pool(name="xin", bufs=6) as xpool,
            tc.tile_pool(name="xsplit", bufs=6) as hpool,
            tc.tile_pool(name="evg", bufs=2) as epool,
            tc.tile_pool(name="ccrep", bufs=2) as rpool,
            tc.tile_pool(name="apsum", bufs=6, space="PSUM") as ppool,
            tc.tile_pool(name="cpsum", bufs=2, space="PSUM") as cpool_ps,
            tc.tile_pool(name="small", bufs=1) as spool,
            tc.tile_pool(name="gath", bufs=1) as gpool,
        ):
            nc.gpsimd.load_library(library_config.mlp)

            Td = dpool.tile([T + 1, D], F32)
            Tv = Td[:][:T, :].rearrange("(c q) d -> c q d", q=Q)

            l128 = cpool.tile([128, 128], F32)
            nc.sync.dma_start(l128[:], L128d)
            l128b = cpool.tile([128, 128], BF16)
            nc.vector.tensor_copy(out=l128b[:], in_=l128[:])
            l32 = cpool.tile([32, 32], F32)
            nc.sync.dma_start(l32[:], L32d)

            gbi = spool.tile([128, S // 16], I16)
            nc.sync.dma_start(gbi[:], GBd)
            gei = spool.tile([128, S // 16], I16)
            nc.sync.dma_start(gei[:], GEd)
            rcp = spool.tile([128, J], F32)
            nc.sync.dma_start(rcp[:], RCPd.rearrange("(j p) -> p j", p=128))

            s32 = spool.tile([32, D], F32)
            nc.vector.memset(s32[:], 0.0)
            srow = spool.tile([1, Q, D], F32)
            ccrow = spool.tile([1, Q, D], F32)
            zrow = spool.tile([1, D], F32)
            nc.vector.memset(zrow[:], 0.0)
            nc.scalar.dma_start(Td[T:T + 1, :], zrow[:])

            Fv = Fd.rearrange("(q r) d -> r q d", r=128)

            def emit_gen_matmuls(g):
                evg = epool.tile([128, CPG * 2, D], F32)
                for cc in range(CPG):
                    c = g * CPG + cc
                    xt = xpool.tile([128, 2, D], F32)
                    nc.sync.dma_start(xt[:], Fv[:, 2 * c:2 * c + 2, :])
                    pt = ppool.tile([128, 2, D], F32, space="PSUM")
                    if SPLIT_BF16:
                        xh = hpool.tile([128, 2, D], BF16)
                        nc.scalar.copy(xh[:], xt[:])
                        xl = hpool.tile([128, 2, D], BF16)
                        nc.vector.tensor_tensor(
                            out=xl[:], in0=xt[:], in1=xh[:],
                            op=mybir.AluOpType.subtract,
                        )
                        nc.tensor.matmul(
                            pt[:], lhsT=l128b[:], rhs=xh[:],
                            start=True, stop=False,
                        )
                        nc.tensor.matmul(
                            pt[:], lhsT=l128b[:], rhs=xl[:],
                            start=False, stop=True,
                        )
                    else:
                        nc.tensor.matmul(
                            pt[:], lhsT=l128[:], rhs=xt[:], start=True, stop=True
                        )
                    # block sums land in psum partition 0 (ones column)
                    nc.scalar.copy(srow[0:1, 2 * c:2 * c + 2, :], pt[0:1, :, :])
                    nc.scalar.dma_start(
                        s32[2 * c:2 * c + 2, :], srow[0:1, 2 * c:2 * c + 2, :]
                    )
                    # prompt eviction frees the psum bank
                    if cc % 2 == 0:
                        nc.vector.tensor_copy(
                            out=evg[:, 2 * cc:2 * cc + 2, :], in_=pt[:]
                        )
                    else:
                        nc.scalar.copy(evg[:, 2 * cc:2 * cc + 2, :], pt[:])
                return evg

            def emit_gen_cc_and_evict(g, evg):
                ccp = cpool_ps.tile([8, D], F32, space="PSUM")
                nc.tensor.matmul(
                    ccp[:], lhsT=l32[:, 8 * g:8 * (g + 1)], rhs=s32[:],
                    start=True, stop=True,
                )
                cc8 = spool.tile([8, D], F32, name=f"cc8_{g}")
                nc.vector.tensor_copy(out=cc8[:], in_=ccp[:])
                nc.scalar.dma_start(ccrow[0:1, 8 * g:8 * (g + 1), :], cc8[:])
                ccrep = rpool.tile([128, CPG * 2, D], F32)
                nc.gpsimd.partition_broadcast(
                    ccrep[:], ccrow[0:1, 8 * g:8 * (g + 1), :]
                )
                nc.vector.tensor_tensor(
                    out=evg[:], in0=evg[:], in1=ccrep[:],
                    op=mybir.AluOpType.add,
                )
                nc.scalar.dma_start(Tv[:, 8 * g:8 * (g + 1), :], evg[:])

            # software pipeline: CC chain for gen g emitted after gen g+1
            # matmuls; prompt copies free psum banks so nothing deadlocks
            pending = None
            for g in range(GENS):
                evg = emit_gen_matmuls(g)
                if pending is not None:
                    emit_gen_cc_and_evict(g - 1, pending)
                pending = evg
            emit_gen_cc_and_evict(GENS - 1, pending)

            pex = gpool.tile([128, J, D], F32)
            pb = gpool.tile([128, J, D], F32)
            nc.gpsimd.dma_gather(
                out_ap=pb[:], in_ap=Td[:], idxs_ap=gbi[:],
                num_idxs=S, num_idxs_reg=S, elem_size=D,
            )
            nc.gpsimd.dma_gather(
                out_ap=pex[:], in_ap=Td[:], idxs_ap=gei[:],
                num_idxs=S, num_idxs_reg=S, elem_size=D,
            )
            nc.vector.tensor_tensor(
                out=pex[:], in0=pex[:], in1=pb[:], op=mybir.AluOpType.subtract
            )

            res = gpool.tile([128, J, D], F32)
            for j in range(J):
                # scaled copy on the scalar engine: out = in * rcp[p]
                nc.scalar.mul(res[:, j, :], pex[:, j, :], rcp[:, j:j + 1])
            # span s = col*128 + partition
            nc.sync.dma_start(OUTd.rearrange("(j p) d -> p j d", p=128), res[:])

    nc.compile()
    return nc


def _get_nc():
    if "nc" not in _CACHE:
        _CACHE["nc"] = _build()
        _CACHE["consts"] = _host_constants()
    return _CACHE["nc"], _CACHE["consts"]


def _prep_indices(b, e):
    b = np.clip(b, 0, T)
    e = np.clip(e, 0, T)
    e = np.maximum(e, b)           # empty window -> diff 0
    e = np.minimum(e, b + MAX_W)   # reference only sums MAX_W positions
    gbi = _permute_idx(b.astype(np.int64))
    gei = _permute_idx(e.astype(np.int64))
    rcp = (1.0 / np.maximum(e - b, 1)).astype(np.float32)
    return gbi, gei, rcp


def kernel(features, begins, ends, _trace=False, _trace_kwargs=None):
    features = np.ascontiguousarray(np.asarray(features, dtype=np.float32))
    b = np.asarray(begins).astype(np.int64)
    e = np.asarray(ends).astype(np.int64)
    Bn = features.shape[0]
    assert features.shape == (Bn, T, D) and Bn == 8

    gbi, gei, rcp = _prep_indices(b, e)

    nc, consts = _get_nc()
    in_maps = []
    for i in range(Bn):
        m = {
            "features": features[i],
            "gbi": _wrap_idx16(gbi[i]),
            "gei": _wrap_idx16(gei[i]),
            "rcp": np.ascontiguousarray(rcp[i]),
        }
        m.update(consts)
        in_maps.append(m)
    kw = {}
    if _trace:
        tk = dict(_trace_kwargs or {})
        tmpdir = tk.pop("tmpdir", None)
        kw = {"trace": True, "trace_kwargs": tk}
        if tmpdir:
            kw["tmpdir"] = tmpdir
    res = run_bass_kernel_spmd(nc, in_maps, list(range(Bn)), **kw)
    out = np.stack([res.results[i]["out"] for i in range(Bn)])
    if _trace:
        return out, res
    return out
